# revision 21
# baseline (speedup 1.0000x reference)
"""Multi-headed causal attention on 8 trn2 NeuronCores (Bass/Tile).

Sharding: tensor-parallel over heads — 2 heads per core, all 4 batches.

Schedule (v3):
  - Tiny warmup AllToAll at t=0 absorbs cross-core launch skew so real
    collectives see a ~1us barrier instead of ~20us.
  - All QKV projections upfront (PE-dense, bf16). et comes in as one
    small j4=0 burst for batch 0 plus two large strided DMAs per batch
    (few descriptor builds, full aggregate DMA bandwidth). V transposes
    are deferred one j4 step so they never wait on the ACT copy.
  - Attention j-outer: groups (j=0 all b, j=2, j=1, j=3) under a single
    global software pipeline (scores run PIPE=4 m-iterations ahead of
    AV, across chunk boundaries).
  - Scores per head are K=64 matmuls against the unpadded stacked K^T,
    written to an fp16 PSUM tile [128, 2, 512] (1 bank -> 4 score tiles
    in flight). exp via one strided-AP activation per m; the causal
    triangle is zeroed by a small affine_select on [128, 2, 128].
  - V padded as [V | ones*64]: softmax denominators land replicated on
    PSUM partitions 64..127; they are bridged to partitions 0..63 by a
    sync-queue DMA (off the busy ACT queue), then reciprocal+mul on DVE.
  - AV skips fully-masked leading columns of diagonal tiles.
  - 4 AllToAlls (one per j-group, at 25/50/75/100% of attention), 256
    output rows per core per group. Out-projection runs as 4-matmul
    micro-tasks injected between attention m-iterations; only the last
    group's out-projection trails the final a2a.
"""
import sys

sys.path.insert(0, "/opt/trn_rl_repo")

import numpy as np

import concourse.bass as bass
import concourse.tile as tile
from concourse import bacc, mybir
from concourse.bass_utils import run_bass_kernel_spmd

B, S, D, H, HD = 4, 2048, 1024, 16, 64
NC_ = 8          # cores
PH = 2           # heads per core
SC = 512         # s_q chunk
NK = S // 128    # 16 s_k chunks of 128
ND = D // 128    # 8 contraction chunks of 128
JORDER = (0, 2, 1, 3)          # j-group processing order
F32 = mybir.dt.float32
FP16 = mybir.dt.float16
BF16 = mybir.dt.bfloat16
EXP = mybir.ActivationFunctionType.Exp
GE = mybir.AluOpType.is_ge
PIPE = 3         # scores lookahead (m-iterations) ahead of AV
FILLER_DELAY = 12   # m-iterations between collective emission and filler


def build():
    nc = bacc.Bacc("TRN2", target_bir_lowering=False, debug=False, num_devices=NC_)

    emb_t = nc.dram_tensor("embedded_t", [B, D, S], BF16, kind="ExternalInput").ap()
    w_qkv = nc.dram_tensor("w_qkv", [3, ND, 128, 128], BF16, kind="ExternalInput").ap()
    wo_t = nc.dram_tensor("wo_t", [ND, 128, D], BF16, kind="ExternalInput").ap()
    bo_row = nc.dram_tensor("bo_row", [1, D], F32, kind="ExternalInput").ap()
    out_shard = nc.dram_tensor("out_shard", [1024, D], F32, kind="ExternalOutput").ap()

    with tile.TileContext(nc) as tc:
        _build_body(nc, tc, emb_t, w_qkv, wo_t, bo_row, out_shard)

    nc.compile()
    return nc


def _build_body(nc, tc, emb_t, w_qkv, wo_t, bo_row, out_shard):
    from contextlib import ExitStack

    ctx = ExitStack()
    with ctx:
        const = ctx.enter_context(tc.tile_pool(name="const", bufs=1))
        # PSUM banks: psA 2x2 (scores/proj), psC 2x1 (ctx), psX 2x1 (po)
        psA = ctx.enter_context(tc.tile_pool(name="psA", bufs=2, space="PSUM"))
        psC = ctx.enter_context(tc.tile_pool(name="psC", bufs=2, space="PSUM"))
        psX = ctx.enter_context(tc.tile_pool(name="psX", bufs=2, space="PSUM"))
        dram = ctx.enter_context(tc.tile_pool(name="dram", bufs=1, space="DRAM"))

        etp = ctx.enter_context(tc.tile_pool(name="etp", bufs=2))
        et0p = ctx.enter_context(tc.tile_pool(name="et0p", bufs=8))
        qtp = ctx.enter_context(tc.tile_pool(name="qtp", bufs=4))
        ktp = ctx.enter_context(tc.tile_pool(name="ktp", bufs=4))
        vtp = ctx.enter_context(tc.tile_pool(name="vtp", bufs=1))
        vsb = ctx.enter_context(tc.tile_pool(name="vsb", bufs=4))
        exp_p = ctx.enter_context(tc.tile_pool(name="exp_p", bufs=6))
        rc_p = ctx.enter_context(tc.tile_pool(name="rc_p", bufs=3))
        cn_p = ctx.enter_context(tc.tile_pool(name="cn_p", bufs=6))
        cat_p = ctx.enter_context(tc.tile_pool(name="cat_p", bufs=2))
        ob_p = ctx.enter_context(tc.tile_pool(name="ob_p", bufs=2))

        # ---- warmup collective: absorb launch skew while proj runs ----
        warm_in = dram.tile([NC_, 1, 16], BF16, tag="warm_in", name="warm_in")
        warm_out = dram.tile([NC_, 1, 16], BF16, tag="warm_out", name="warm_out")
        nc.gpsimd.collective_compute(
            "AllToAll", mybir.AluOpType.bypass,
            replica_groups=[list(range(NC_))],
            ins=[warm_in.opt()], outs=[warm_out.opt()])

        # ---- weights first (small), then batch-0 j4=0 burst ----
        wq_all = const.tile([128, 24, 128], BF16, tag="wq_all")
        for p in range(3):
            nc.sync.dma_start(out=wq_all[:, 8 * p:8 * (p + 1), :],
                              in_=bass.AP(
                tensor=w_qkv.tensor, offset=131072 * p,
                ap=[[128, 128], [16384, 8], [1, 128]]))
        wq_sb = [[wq_all[:, 8 * p + c, :] for c in range(ND)] for p in range(3)]

        # batch-0 j4=0: 8 small tiles for a fast first projection
        et0 = {}
        for c in range(ND):
            t = et0p.tile([128, SC], BF16, tag="et0", name=f"et0_{c}")
            eng = nc.sync if (c % 2 == 0) else nc.scalar
            eng.dma_start(out=t[:], in_=emb_t[0, 128 * c:128 * (c + 1), 0:SC])
            et0[c] = t

        # per-batch halves [128, ND, 1024]: j4-pair (0,1) and (2,3)
        # batch 0 half0 covers only j4=1 (j4=0 came via et0)
        halves = {}

        def fetch_half(b, hidx):
            s0 = SC if (b == 0 and hidx == 0) else 2 * SC * hidx
            s1 = 2 * SC * (hidx + 1)
            t = etp.tile([128, ND, 2 * SC], BF16, tag="eth",
                         name=f"eth{b}_{hidx}")
            eng = nc.sync if hidx == 0 else nc.scalar
            eng.dma_start(
                out=t[:, :, s0 - 2 * SC * hidx:s1 - 2 * SC * hidx],
                in_=bass.AP(
                    tensor=emb_t.tensor, offset=b * D * S + s0,
                    ap=[[S, 128], [128 * S, ND], [1, s1 - s0]]))
            halves[(b, hidx)] = t

        def et_ap(b, j4, c):
            if b == 0 and j4 == 0:
                return et0[c][:]
            t = halves[(b, j4 // 2)]
            lo = SC * (j4 % 2)
            return t[:, c, lo:lo + SC]

        fetch_half(0, 0)
        fetch_half(0, 1)

        bo_sb = const.tile([1, D], F32, tag="bo1")
        nc.sync.dma_start(out=bo_sb[:], in_=bo_row[:])
        bo_b = const.tile([128, D], F32, tag="bob")
        nc.gpsimd.partition_broadcast(bo_b[:], bo_sb[:])

        ident = const.tile([128, 128], BF16, tag="ident")
        nc.gpsimd.memset(ident[:], 1.0)
        nc.gpsimd.affine_select(out=ident[:], in_=ident[:], compare_op=GE,
                                fill=0.0, base=0, pattern=[[-1, 128]],
                                channel_multiplier=1)
        nc.gpsimd.affine_select(out=ident[:], in_=ident[:], compare_op=GE,
                                fill=0.0, base=0, pattern=[[1, 128]],
                                channel_multiplier=-1)

        wot_sb = [const.tile([128, D], BF16, tag=f"wo{c}", name=f"wo{c}")
                  for c in range(ND)]
        for c in range(ND):
            nc.scalar.dma_start(out=wot_sb[c][:], in_=wo_t[c])

        # a2a buffers: one per j-group; block to peer o = [128, 256]
        a2a_in = [dram.tile([NC_, 128, 256], BF16, tag=f"a2a_in{g}",
                            name=f"a2a_in{g}") for g in range(4)]
        a2a_out = [dram.tile([NC_, 128, 256], BF16, tag=f"a2a_out{g}",
                             name=f"a2a_out{g}") for g in range(4)]

        # ---- projection phase: all 4 batches ----
        ps_rot = [0]

        def next_ps(shape, dtype, name):
            i = ps_rot[0] % 4
            ps_rot[0] += 1
            pool, tg = (psA, "A") if i < 2 else (psX, "X")
            return pool.tile(shape, dtype, tag=tg, name=name)

        qt, kt, v01 = {}, {}, {}
        for b in range(B):
            qt[b] = qtp.tile([128, S], BF16, tag="qt", name=f"qt{b}")
            kt[b] = ktp.tile([128, S], BF16, tag="kt", name=f"kt{b}")
            vt = vtp.tile([128, S], BF16, tag="vt", name=f"vt{b}")
            v01[b] = [vsb.tile([128, NK, 128], BF16, tag=f"v{h}",
                               name=f"v{b}_{h}") for h in range(PH)]
            for h in range(PH):
                nc.vector.memset(v01[b][h][:, :, 64:128], 1.0)


            def emit_tr(g4):
                pt = next_ps([128, 4, 128], BF16, f"tr{b}_{g4}")
                for i in range(4):
                    sk = 4 * g4 + i
                    nc.tensor.transpose(pt[:, i, :],
                                        vt[:, 128 * sk:128 * (sk + 1)],
                                        ident[:])
                for h in range(PH):
                    nc.vector.tensor_copy(
                        v01[b][h][:, 4 * g4:4 * (g4 + 1), 0:64],
                        pt[:, :, 64 * h:64 * (h + 1)])

            for j4 in range(4):
                # prefetch next batch once this batch's first half is done
                if j4 == 2 and b + 1 < B:
                    fetch_half(b + 1, 0)
                    fetch_half(b + 1, 1)
                sl = slice(SC * j4, SC * (j4 + 1))
                for p in range(3):
                    ps = next_ps([128, SC], F32, f"pj{b}_{j4}_{p}")
                    for c in range(ND):
                        nc.tensor.matmul(
                            ps[:], lhsT=wq_sb[p][c], rhs=et_ap(b, j4, c),
                            start=(c == 0), stop=(c == ND - 1))
                    if p == 0:
                        nc.vector.tensor_copy(qt[b][:, sl], ps[:])
                    elif p == 1:
                        nc.vector.tensor_copy(kt[b][:, sl], ps[:])
                    else:
                        nc.scalar.copy(vt[:, sl], ps[:])
                if j4 > 0:
                    emit_tr(j4 - 1)   # vt cols of j4-1 are long since ready
            emit_tr(3)

        # ---- attention phase: global pipeline over 16 chunks ----
        chunks = [(b, j) for j in JORDER for b in range(B)]
        pending = []
        filler = []         # (ready_mctr, thunk)
        mctr = [0]

        def emit_scores(b, j, m, ctx_pair):
            c0 = max(0, 128 * m - SC * j)
            psc = psA.tile([128, PH, SC], F32, tag="A", name=f"sc{b}_{j}_{m}")
            for h in range(PH):
                nc.tensor.matmul(
                    psc[:, h, c0:SC],
                    lhsT=kt[b][64 * h:64 * (h + 1), 128 * m:128 * (m + 1)],
                    rhs=qt[b][64 * h:64 * (h + 1), SC * j + c0:SC * (j + 1)],
                    start=True, stop=True)
            ex = exp_p.tile([128, PH, SC], BF16, tag="ex",
                            name=f"ex{b}_{j}_{m}")
            nc.scalar.activation(out=ex[:, :, c0:], in_=psc[:, :, c0:],
                                 func=EXP, scale=0.125)
            if m >= 4 * j:  # diagonal tile: zero k>q entries in the 128 block
                nc.gpsimd.affine_select(
                    out=ex[:, :, c0:c0 + 128], in_=ex[:, :, c0:c0 + 128],
                    compare_op=GE, fill=0.0, base=0,
                    pattern=[[0, PH], [1, 128]], channel_multiplier=-1)
            return ex

        def emit_av(item):
            b, j, m, ex, is_last, ctx_pair = item
            c0 = max(0, 128 * m - SC * j)
            for h in range(PH):
                nc.tensor.matmul(
                    ctx_pair[h][:, c0:SC], lhsT=v01[b][h][:, m, :],
                    rhs=ex[:, h, c0:SC],
                    start=(m == 0), stop=is_last)
            if is_last:
                finish_chunk(b, j, ctx_pair)

        def finish_chunk(b, j, ctx_pair):
            g = JORDER.index(j)
            for h in range(PH):
                # bridge replicated denominators to partitions 0..63 (ACT
                # is the only engine that can shift partitions from PSUM)
                dn = rc_p.tile([64, SC], F32, tag="dn")
                nc.scalar.copy(dn[:], ctx_pair[h][64:128, :])
                rc = rc_p.tile([64, SC], F32, tag="rc")
                nc.vector.reciprocal_approx_fast(rc[:], dn[:])
                cn = cn_p.tile([64, SC], BF16, tag="cn")
                nc.vector.tensor_mul(cn[:], ctx_pair[h][0:64, :], rc[:])
                for hf in range(2):
                    nc.scalar.dma_start(
                        out=a2a_in[g][2 * b + hf, 64 * h:64 * (h + 1), :],
                        in_=cn[:, 256 * hf:256 * (hf + 1)])
            if b == B - 1:      # last chunk of the group -> fire collective
                nc.gpsimd.collective_compute(
                    "AllToAll", mybir.AluOpType.bypass,
                    replica_groups=[list(range(NC_))],
                    ins=[a2a_in[g].opt()], outs=[a2a_out[g].opt()])
                emit_cats_and_queue_outproj(g)

        def emit_cats_and_queue_outproj(g):
            cats = []
            for r in range(NC_):
                ct = cat_p.tile([128, 256], BF16, tag=f"cat{r}",
                                name=f"cat{g}_{r}")
                nc.sync.dma_start(out=ct[:], in_=a2a_out[g][r])
                cats.append(ct)
            for s2 in range(2):
                for n in range(2):
                    po_box = []
                    for kh in range(2):
                        filler.append((
                            mctr[0] + FILLER_DELAY,
                            _mk_outproj_micro(g, s2, n, kh, cats, po_box)))

        def _mk_outproj_micro(g, s2, n, kh, cats, po_box):
            def thunk():
                if not po_box:
                    po_box.append(psX.tile([128, SC], F32, tag="X",
                                           name=f"po{g}_{s2}_{n}"))
                po = po_box[0]
                for kp in range(4 * kh, 4 * kh + 4):
                    nc.tensor.matmul(
                        po[:],
                        lhsT=cats[kp][:, 128 * s2:128 * (s2 + 1)],
                        rhs=wot_sb[kp][:, SC * n:SC * (n + 1)],
                        start=(kp == 0), stop=(kp == ND - 1))
                if kh == 1:
                    ob = ob_p.tile([128, SC], F32, tag="ob")
                    nc.vector.tensor_add(ob[:], po[:],
                                         bo_b[:, SC * n:SC * (n + 1)])
                    r0 = 256 * g + 128 * s2
                    nc.sync.dma_start(
                        out=out_shard[r0:r0 + 128, SC * n:SC * (n + 1)],
                        in_=ob[:])
            return thunk

        for b, j in chunks:
            mtop = 4 * j + 4
            ctx_pair = [psC.tile([128, SC], F32, tag="ctx",
                                 name=f"ctx{b}_{j}_{h}") for h in range(PH)]
            for m in range(mtop):
                ex = emit_scores(b, j, m, ctx_pair)
                pending.append((b, j, m, ex, m == mtop - 1, ctx_pair))
                if len(pending) > PIPE:
                    emit_av(pending.pop(0))
                mctr[0] += 1
                if filler and mctr[0] >= filler[0][0]:
                    filler.pop(0)[1]()
        while pending:
            emit_av(pending.pop(0))
        while filler:
            filler.pop(0)[1]()


_NC_CACHE = None


def _get_nc():
    global _NC_CACHE
    if _NC_CACHE is None:
        _NC_CACHE = build()
    return _NC_CACHE


def kernel(embedded, Wq, Wk, Wv, Wo, bo, _trace=False):
    import ml_dtypes
    embedded = np.ascontiguousarray(np.asarray(embedded, np.float32))
    emb_t = np.ascontiguousarray(embedded.transpose(0, 2, 1)).astype(
        ml_dtypes.bfloat16)
    W = np.stack([np.asarray(Wq), np.asarray(Wk), np.asarray(Wv)]).astype(
        np.float32)
    wo_t = np.ascontiguousarray(np.asarray(Wo, np.float32).T).astype(
        ml_dtypes.bfloat16).reshape(ND, 128, D)
    bo_row = np.asarray(bo, np.float32).reshape(1, D)

    in_maps = []
    for c in range(NC_):
        w = W[:, 2 * c:2 * c + 2]                  # [3, 2, D, HD]
        w = np.ascontiguousarray(w.transpose(0, 2, 1, 3)).reshape(
            3, ND, 128, 128).astype(ml_dtypes.bfloat16)
        in_maps.append({
            "embedded_t": emb_t,
            "w_qkv": w,
            "wo_t": wo_t,
            "bo_row": bo_row,
        })

    nc = _get_nc()
    res = run_bass_kernel_spmd(nc, in_maps, core_ids=list(range(NC_)),
                               trace=_trace)

    out = np.empty((B, S, D), np.float32)
    for c in range(NC_):
        r = res.results[c]["out_shard"]            # [1024, D]
        for gi, j in enumerate(JORDER):
            for s2 in range(2):
                s0 = SC * j + 256 * (c % 2) + 128 * s2
                out[c // 2, s0:s0 + 128, :] = \
                    r[256 * gi + 128 * s2:256 * gi + 128 * s2 + 128]
    if _trace:
        return out, res
    return out
